# revision 4
# baseline (speedup 1.0000x reference)
"""CoAttention kernel for Trainium2 (8 NeuronCores, data-parallel over batch).

Math (per sample): ta = relu(seq_a @ W + b), tb likewise.  The reference
mean-pools the [N, rv_len, M] affinity before softmax, and mean-pooling
commutes with the dot product:

    atob_scores[n, l] = mean_m( ta[n,l,:] . tb_all_tokens[m,:] )
                      = ta[n,l,:] . mean_m( tb_all_tokens[m,:] )

so each side only needs a dot with the *other side's per-sample mean
feature vector* — the 52M-element affinity tensor is never materialized.

Schedule: the kernel is HBM-bound (12.3 MB of seq data per core, ~37 us
at full DMA rate), so all 24 seq tiles are DMA'd up front, round-robin
over four engine queues, with enough SBUF buffers (3 samples deep) that
the stream never stalls on consumption.  The PE pipeline (FC matmuls,
then per-sample score matvecs one sample behind) chases the stream;
relu evictions (scalar), softmax (DVE), weight broadcast (gpsimd) and
the weighted-sum multiply+segmented-reduce (DVE) all trail behind on
their own engines.  Score matvecs write bank-sized PSUM chunks that are
copied+reshaped straight into the [reviews, rv_len] softmax layout.
"""
import sys

sys.path.insert(0, "/opt/trn_rl_repo")

import numpy as np

import concourse.bacc as bacc
import concourse.tile as tile
from concourse import mybir

# Problem shape (hardcoded per contest contract)
BZ, RV, RL, DIN, DH = 32, 10, 128, 300, 128
NCORES = 8
BPC = BZ // NCORES            # samples per core: 4
TPC = BPC * RV * RL           # tokens per core per side: 5120
TPS = RV * RL                 # tokens per sample: 1280
RPC = BPC * RV                # reviews per core: 40
NEG_INF = -1e9

f32 = mybir.dt.float32
f32r = mybir.dt.float32r
i32 = mybir.dt.int32
AF = mybir.ActivationFunctionType
AX = mybir.AxisListType

# d-chunks of the contraction dim (K <= 128)
DCH = [(0, 128), (128, 128), (256, 44)]
# free-dim chunks of one sample's tokens (N <= 512, >= 256 for fast fp32r)
NCH = [(0, 512), (512, 512), (1024, 256)]

_CACHE = {}


def _build(iters=1, serial=False, loop_n=0, stage=3):
    nc = bacc.Bacc("TRN2", target_bir_lowering=False, debug=False)

    sqt = {s: nc.dram_tensor(f"sqt_{s}", [DIN, TPC], f32r, kind="ExternalInput")
           for s in "ab"}
    msk2_d = nc.dram_tensor("msk2", [2 * RV, BPC * RL], i32,
                            kind="ExternalInput")
    w_d = nc.dram_tensor("w", [DIN, DH], f32r, kind="ExternalInput")
    bias_d = nc.dram_tensor("bias", [DH, 1], f32, kind="ExternalInput")
    ident_d = nc.dram_tensor("ident", [DH, DH], f32, kind="ExternalInput")

    out_v = {s: nc.dram_tensor(f"out_{s}", [RPC, DH], f32, kind="ExternalOutput")
             for s in "ab"}
    out_w = {s: nc.dram_tensor(f"outw_{s}", [RPC, RL], f32, kind="ExternalOutput")
             for s in "ab"}

    import contextlib
    outer_tc = tile.TileContext(nc) if not serial else None
    with (outer_tc if outer_tc is not None else contextlib.nullcontext()):
      for it_ in range(iters):
        pfx = f"i{it_}_" if iters > 1 else ""
        with (
            tile.TileContext(nc) if serial else contextlib.nullcontext()
        ) as maybe_tc:
          tc = maybe_tc if serial else outer_tc
          with (
            tc.For_i(0, loop_n, 1) if loop_n else contextlib.nullcontext()
          ):
           with (
            tc.tile_pool(name=pfx + "cst", bufs=1) as cst,
            tc.tile_pool(name=pfx + "seq", bufs=18) as seqp,
            tc.tile_pool(name=pfx + "big", bufs=1) as bigp,
            tc.tile_pool(name=pfx + "sm", bufs=2) as smp_pool,
            tc.tile_pool(name=pfx + "ps", bufs=2, space="PSUM") as ps,
        ):
            # constants go on the scalar queue ahead of its share of the
            # seq stream (weights are needed by the very first matmul)
            w_t = {}
            for c, (d0, dw) in enumerate(DCH):
                w_t[c] = cst.tile([dw, DH], f32r, tag=f"w{c}", name=f"{pfx}w_t{c}")
                nc.scalar.dma_start(w_t[c][:], w_d[d0:d0 + dw, :])
            bias_t = cst.tile([DH, 1], f32, tag="bias", name=pfx + "bias_t")
            nc.scalar.dma_start(bias_t[:], bias_d[:])
            msk_t2 = cst.tile([2 * RV, BPC * RL], i32, tag="msk2", name=pfx + "msk_t2")
            ident_t = cst.tile([DH, DH], f32, tag="ident", name=pfx + "ident_t")

            # ---- the full seq stream, issued up front in consumption
            # order, round-robin over four queues.  18 bufs = 3 samples
            # in flight, so the reuse wait (sample k+3 behind sample k's
            # last matmul) never gates the stream in practice.
            qs = [nc.sync, nc.gpsimd, nc.scalar]
            sq = {}
            qi = 0
            for smp in range(BPC):
                t0 = smp * TPS
                for c, (d0, dw) in enumerate(DCH):
                    for s in ("b", "a"):
                        tl = seqp.tile([dw, TPS], f32r, tag="seq",
                                       name=f"{pfx}sq_{s}{smp}{c}")
                        qs[qi % len(qs)].dma_start(
                            tl[:], sqt[s][d0:d0 + dw, t0:t0 + TPS])
                        sq[(s, c, smp)] = tl
                        qi += 1
                if smp == 0:
                    # mask is first needed by tail(0); ident only at the
                    # epilogue — keep both off the critical stream head
                    nc.sync.dma_start(msk_t2[:], msk2_d[:])
            nc.gpsimd.dma_start(ident_t[:], ident_d[:])

            taT, acc, mean, aoutT = {}, {}, {}, {}
            for s in "ab":
                taT[s] = bigp.tile([DH, TPC], f32r, tag=f"taT{s}",
                                   name=f"{pfx}taT_{s}")
                acc[s] = cst.tile([DH, BPC], f32, tag=f"acc{s}", name=f"{pfx}acc_{s}")
                mean[s] = cst.tile([DH, BPC], f32r, tag=f"mean{s}",
                                   name=f"{pfx}mean_{s}")
                aoutT[s] = cst.tile([DH, RPC], f32, tag=f"aoutT{s}",
                                    name=f"{pfx}aoutT_{s}")

            other = {"a": "b", "b": "a"}
            w2d_tiles = {}
            for smp in range(BPC):
                w2d_tiles[smp] = cst.tile(
                    [2 * RV, RL], f32, tag=f"w2d{smp}",
                    name=f"{pfx}w2ds_{smp}")

            def emit_fc_pair(smp):
                if stage < 1:
                    return
                t0 = smp * TPS
                pfc = {}
                for s in ("b", "a"):
                    pfc[s] = ps.tile([DH, TPS], f32, tag="fc", bufs=2,
                                     name=f"{pfx}pfc_{s}{smp}")
                # c-outer: 3 weight loads per sample pair instead of 18
                for c in range(3):
                    for s in ("b", "a"):
                        for n0, nw in NCH:
                            nc.tensor.matmul(
                                pfc[s][:, n0:n0 + nw],
                                w_t[c][:],
                                sq[(s, c, smp)][:, n0:n0 + nw],
                                start=(c == 0), stop=(c == 2))
                for s in ("b", "a"):
                    nc.scalar.activation(
                        taT[s][:, t0:t0 + TPS], pfc[s][:], AF.Relu,
                        bias=bias_t[:], accum_out=acc[s][:, smp:smp + 1])
                    nc.scalar.mul(mean[s][:, smp:smp + 1],
                                  acc[s][:, smp:smp + 1], 1.0 / TPS)

            def emit_tail(smp):
                if stage < 2:
                    return
                t0 = smp * TPS
                # scores: M=1 matvec against the other side's mean, in
                # bank-sized PSUM chunks evicted straight into the
                # [reviews, rv_len] softmax layout
                scs = smp_pool.tile([2 * RV, RL], f32, tag="scs", bufs=3,
                                    name=f"{pfx}scs_{smp}")
                for i, s in enumerate(("a", "b")):
                    for ci, (n0, nw) in enumerate(NCH):
                        pscc = ps.tile([1, 512], f32, tag="sc", bufs=2,
                                       name=f"{pfx}psc_{s}{smp}{ci}")
                        nc.tensor.matmul(
                            pscc[:, :nw],
                            mean[other[s]][:, smp:smp + 1],
                            taT[s][:, t0 + n0:t0 + n0 + nw])
                        src = smp_pool.tile([1, 512], f32, tag="srowc", bufs=4,
                                            name=f"{pfx}src_{s}{smp}{ci}")
                        nc.scalar.copy(src[:, :nw], pscc[:, :nw])
                        r0 = i * RV + n0 // RL
                        nc.sync.dma_start(scs[r0:r0 + nw // RL, :],
                                          src[:, :nw])

                # masked softmax for both sides' reviews (a rows 0-9, b 10-19)
                lgs = smp_pool.tile([2 * RV, RL], f32, tag="lgs", bufs=3,
                                    name=f"{pfx}lgs_{smp}")
                nc.vector.memset(lgs[:], NEG_INF)
                nc.vector.copy_predicated(
                    lgs[:], msk_t2[:, smp * RL:(smp + 1) * RL], scs[:])
                negmax = smp_pool.tile([2 * RV, 1], f32, tag="negmax", bufs=3,
                                       name=f"{pfx}negmax_{smp}")
                nc.vector.reduce_max(out=negmax[:], in_=lgs[:],
                                     axis=AX.X, negate=True)
                e2d = smp_pool.tile([2 * RV, RL], f32, tag="e2d", bufs=3,
                                    name=f"{pfx}e2d_{smp}")
                ssum = smp_pool.tile([2 * RV, 1], f32, tag="ssum", bufs=3,
                                     name=f"{pfx}ssum_{smp}")
                nc.scalar.activation(e2d[:], lgs[:], AF.Exp, bias=negmax[:],
                                     accum_out=ssum[:])
                rec = smp_pool.tile([2 * RV, 1], f32, tag="rec", bufs=3,
                                    name=f"{pfx}rec_{smp}")
                nc.vector.reciprocal(rec[:], ssum[:])
                w2ds = w2d_tiles[smp]
                nc.vector.tensor_scalar_mul(w2ds[:], e2d[:], rec[:])

                # weighted sums
                for i, s in enumerate(("a", "b") if stage >= 3 else ()):
                    wflat = smp_pool.tile([1, TPS], f32, tag="wflat", bufs=2,
                                          name=f"{pfx}wflat_{s}{smp}")
                    nc.gpsimd.dma_start(
                        wflat[:], w2ds[i * RV:(i + 1) * RV, :])
                    wbc = smp_pool.tile([DH, TPS], f32, tag="wbc", bufs=2,
                                        name=f"{pfx}wbc_{s}{smp}")
                    nc.gpsimd.partition_broadcast(wbc[:], wflat[:])
                    tmp = smp_pool.tile([DH, TPS], f32, tag="tmp", bufs=2,
                                        name=f"{pfx}tmp_{s}{smp}")
                    nc.vector.tensor_tensor(
                        out=tmp[:], in0=taT[s][:, t0:t0 + TPS].bitcast(f32),
                        in1=wbc[:], op=mybir.AluOpType.mult)
                    nc.vector.reduce_sum(
                        out=aoutT[s][:, smp * RV:(smp + 1) * RV],
                        in_=tmp[:].rearrange("p (r l) -> p r l", r=RV),
                        axis=AX.X)

            # FC runs one sample ahead of the score/softmax/weighted-sum tail
            # so the in-order PE queue never stalls on an eviction.
            for smp in range(BPC):
                emit_fc_pair(smp)
                if smp >= 1:
                    emit_tail(smp - 1)
            emit_tail(BPC - 1)

            # ---- per-side epilogue: weights out, transpose, vectors out
            for si, s in enumerate(("a", "b") if stage >= 2 else ()):
                for smp in range(BPC):
                    nc.sync.dma_start(
                        out_w[s][smp * RV:(smp + 1) * RV, :],
                        w2d_tiles[smp][si * RV:(si + 1) * RV, :])
                ptp = ps.tile([RPC, DH], f32, tag="sc", bufs=2,
                              name=f"{pfx}ptp_{s}")
                nc.tensor.matmul(ptp[:], aoutT[s][:], ident_t[:],
                                 is_transpose=True)
                aout = smp_pool.tile([RPC, DH], f32, tag="aout",
                                     name=f"{pfx}aout_{s}")
                nc.vector.tensor_copy(aout[:], ptp[:])
                nc.sync.dma_start(out_v[s][:], aout[:])

    nc.compile()
    return nc


def build_in_maps(seq_a, seq_b, mask_a, mask_b, W, b):
    seq_a = np.asarray(seq_a, dtype=np.float32)
    seq_b = np.asarray(seq_b, dtype=np.float32)
    mask_a = np.asarray(mask_a, dtype=np.int32)
    mask_b = np.asarray(mask_b, dtype=np.int32)
    W = np.asarray(W, dtype=np.float32)
    b = np.asarray(b, dtype=np.float32)

    ident_np = np.eye(DH, dtype=np.float32)
    bias_np = np.ascontiguousarray(b.reshape(DH, 1))
    w_np = np.ascontiguousarray(W)

    in_maps = []
    for core in range(NCORES):
        b0 = core * BPC
        sl = {}
        for name, seq in (("a", seq_a), ("b", seq_b)):
            chunk = seq[b0:b0 + BPC].reshape(TPC, DIN)
            sl[f"sqt_{name}"] = np.ascontiguousarray(chunk.T)
        sl["msk2"] = np.ascontiguousarray(np.concatenate([
            mask[b0:b0 + BPC].reshape(BPC, RV, RL).transpose(1, 0, 2)
            .reshape(RV, BPC * RL) for mask in (mask_a, mask_b)], axis=0))
        sl["w"] = w_np
        sl["bias"] = bias_np
        sl["ident"] = ident_np
        in_maps.append(sl)
    return in_maps


def kernel(seq_a, seq_b, mask_a, mask_b, W, b):
    if "nc" not in _CACHE:
        _CACHE["nc"] = _build()
    nc = _CACHE["nc"]
    in_maps = build_in_maps(seq_a, seq_b, mask_a, mask_b, W, b)

    from concourse.bass_utils import run_bass_kernel_spmd
    res = run_bass_kernel_spmd(nc, in_maps, core_ids=list(range(NCORES)))
    _CACHE["last_result"] = res

    a_out = np.concatenate([r["out_a"] for r in res.results], axis=0)
    b_out = np.concatenate([r["out_b"] for r in res.results], axis=0)
    atob_w = np.concatenate([r["outw_a"] for r in res.results], axis=0)
    btoa_w = np.concatenate([r["outw_b"] for r in res.results], axis=0)
    return (a_out, b_out, atob_w, btoa_w)
